# revision 19
# baseline (speedup 1.0000x reference)
"""Trainium2 Bass kernel for MPMultiHeadAttention (P=4 masks, B=2, L=1024, D=512, H=8).

Sharding: 8 cores = (p, b) pairs; core c handles mask p=c//2, batch b=c%2.
Each core computes all H=8 heads for its (p,b):
  LN(q[b]) -> Q/K/V projections -> scores^T -> exp*mask -> PV (+denominator
  via an appended ones-column on V) -> normalize -> FC.
Per-core outputs: out_u = FC output [1024,512] (host adds fc_b + residual),
attnT = pre-softmax scores, transposed, for 2 heads [2,1024,1024] (host
transposes back). Head ownership is made SPMD-uniform by permuting heads
host-side so each core's owned heads are head-slots 0 and 1.

All activations live in transposed layout (feature dim on partitions) so every
matmul contraction runs without on-device transposes; host pre-transposes
q/k/v/mask and the weights (pure layout prep on the sharded inputs).
"""

import numpy as np
import ml_dtypes

import concourse.bacc as bacc
import concourse.bass as bass
import concourse.tile as tile
from concourse import mybir
from concourse.bass_utils import run_bass_kernel_spmd

B, L, D = 2, 1024, 512
H, DK, DV, P = 8, 64, 64, 4
TEMP = float(DK) ** 0.5
EPS = 1e-6
NCORES = 8

FP32 = mybir.dt.float32
F32R = mybir.dt.float32r
BF16 = mybir.dt.bfloat16

NLT = L // 128          # 8 lq/lk tiles of 128
NDT = D // 128          # 4 d tiles of 128
NET = D // 128          # 4 e tiles (H*DK = 512)

# Profiling knobs (test.py sets TRACE=True; harness leaves it False)
TRACE = False
LAST_EXEC_NS = None
LAST_RESULTS = None

_COMPILED = None


def _emit(ctx, tc):
    nc = tc.nc
    AF = mybir.ActivationFunctionType
    OP = mybir.AluOpType

    # ---- DRAM I/O ----
    qT_d = nc.dram_tensor("qT", [D, L], F32R, kind="ExternalInput").ap()
    kT_d = nc.dram_tensor("kT", [D, L], F32R, kind="ExternalInput").ap()
    vT_d = nc.dram_tensor("vT", [D, L], F32R, kind="ExternalInput").ap()
    maskT_d = nc.dram_tensor("maskT", [L, L], BF16, kind="ExternalInput").ap()
    wqT_d = nc.dram_tensor("wqT", [D, D], F32R, kind="ExternalInput").ap()
    wkT_d = nc.dram_tensor("wkT", [D, D], F32R, kind="ExternalInput").ap()
    wvT_d = nc.dram_tensor("wvT", [D, D], F32R, kind="ExternalInput").ap()
    fcwT_d = nc.dram_tensor("fcwT", [D, D], F32R, kind="ExternalInput").ap()
    lng_d = nc.dram_tensor("ln_g", [D, 1], FP32, kind="ExternalInput").ap()
    lnb_d = nc.dram_tensor("ln_b", [D, 1], FP32, kind="ExternalInput").ap()
    onesc_d = nc.dram_tensor("ones_c", [128, 1], F32R, kind="ExternalInput").ap()
    onesr_d = nc.dram_tensor("ones_r", [1, 128], F32R, kind="ExternalInput").ap()
    outu_d = nc.dram_tensor("out_u", [L, D], FP32, kind="ExternalOutput").ap()
    attnT_d = nc.dram_tensor("attnT", [2, L, L], FP32, kind="ExternalOutput").ap()

    const = ctx.enter_context(tc.tile_pool(name="const", bufs=1))
    sb = ctx.enter_context(tc.tile_pool(name="sb", bufs=1))
    sq_pool = ctx.enter_context(tc.tile_pool(name="sqp", bufs=4))
    et_pool = ctx.enter_context(tc.tile_pool(name="etp", bufs=16))
    stage_pool = ctx.enter_context(tc.tile_pool(name="stp", bufs=2))
    fco_pool = ctx.enter_context(tc.tile_pool(name="fco", bufs=2))
    ps_big = ctx.enter_context(
        tc.tile_pool(name="psb", bufs=2, space=bass.MemorySpace.PSUM))
    ps_small = ctx.enter_context(
        tc.tile_pool(name="pss", bufs=4, space=bass.MemorySpace.PSUM))

    # ---- constants / weights ----
    wq = [const.tile([128, D], F32R, tag=f"wq{i}", name="wq") for i in range(NDT)]
    wk = [const.tile([128, D], F32R, tag=f"wk{i}", name="wk") for i in range(NDT)]
    wv = [const.tile([128, D], F32R, tag=f"wv{i}", name="wv") for i in range(NDT)]
    fcw = [const.tile([128, D], F32R, tag=f"fcw{i}", name="fcw") for i in range(NET)]
    for i in range(NDT):
        nc.sync.dma_start(out=wq[i], in_=wqT_d[i * 128:(i + 1) * 128, :])
        nc.sync.dma_start(out=wk[i], in_=wkT_d[i * 128:(i + 1) * 128, :])
        nc.sync.dma_start(out=wv[i], in_=wvT_d[i * 128:(i + 1) * 128, :])
        nc.sync.dma_start(out=fcw[i], in_=fcwT_d[i * 128:(i + 1) * 128, :])
    g_col = [const.tile([128, 1], FP32, tag=f"gc{i}", name="gc") for i in range(NDT)]
    b_col = [const.tile([128, 1], FP32, tag=f"bc{i}", name="bc") for i in range(NDT)]
    for i in range(NDT):
        nc.sync.dma_start(out=g_col[i], in_=lng_d[i * 128:(i + 1) * 128, :])
        nc.sync.dma_start(out=b_col[i], in_=lnb_d[i * 128:(i + 1) * 128, :])
    ones_col = const.tile([128, 1], F32R, tag="oc")
    nc.sync.dma_start(out=ones_col, in_=onesc_d)
    ones_row = const.tile([1, 128], F32R, tag="orow")
    nc.sync.dma_start(out=ones_row, in_=onesr_d)
    eps_t = const.tile([1, 1], FP32, tag="eps")
    nc.vector.memset(eps_t, EPS)

    # ---- activations ----
    qTt = [sb.tile([128, L], F32R, tag=f"qT{i}", name="qTt") for i in range(NDT)]
    kTt = [sb.tile([128, L], F32R, tag=f"kT{i}", name="kTt") for i in range(NDT)]
    vTt = [sb.tile([128, L], F32R, tag=f"vT{i}", name="vTt") for i in range(NDT)]
    for i in range(NDT):
        for sg in range(2):
            csl = slice(sg * 512, (sg + 1) * 512)
            nc.sync.dma_start(out=qTt[i][:, csl],
                              in_=qT_d[i * 128:(i + 1) * 128, csl])
            nc.sync.dma_start(out=kTt[i][:, csl],
                              in_=kT_d[i * 128:(i + 1) * 128, csl])
            nc.sync.dma_start(out=vTt[i][:, csl],
                              in_=vT_d[i * 128:(i + 1) * 128, csl])
    maskT = [sb.tile([128, L], BF16, tag=f"mask{j}", name="maskT") for j in range(NLT)]
    for j in range(NLT):
        nc.sync.dma_start(out=maskT[j], in_=maskT_d[j * 128:(j + 1) * 128, :])

    # ---- Phase 1: LayerNorm of q, in transposed layout ----
    # col sums via ones-matmuls; mean/var rows; broadcast rows back via matmul
    mu_row = sb.tile([1, L], F32R, tag="mu")
    var_row = sb.tile([1, L], FP32, tag="var")
    rstd_row = sb.tile([1, L], F32R, tag="rstd")
    sqs = [sq_pool.tile([128, L], F32R, tag="sq", name="sq") for _ in range(NDT)]
    for i in range(NDT):
        for sg in range(2):
            csl = slice(sg * 512, (sg + 1) * 512)
            nc.vector.tensor_mul(sqs[i][:, csl], qTt[i][:, csl],
                                 qTt[i][:, csl])
    for seg in range(2):
        sl = slice(seg * 512, (seg + 1) * 512)
        ps1 = ps_small.tile([1, 512], FP32, tag="small")
        for i in range(NDT):
            nc.tensor.matmul(ps1, ones_col, qTt[i][:, sl],
                             start=(i == 0), stop=(i == NDT - 1))
        nc.scalar.mul(mu_row[:, sl], ps1, 1.0 / D)
        ps2 = ps_small.tile([1, 512], FP32, tag="small")
        for i in range(NDT):
            nc.tensor.matmul(ps2, ones_col, sqs[i][:, sl],
                             start=(i == 0), stop=(i == NDT - 1))
        nc.scalar.mul(var_row[:, sl], ps2, 1.0 / D)
    # var = E[x^2] - mu^2 ; rstd = 1/sqrt(var+eps)
    nc.vector.tensor_mul(rstd_row, mu_row, mu_row)       # rstd_row = mu^2 (tmp)
    nc.vector.tensor_sub(var_row, var_row, rstd_row)
    nc.scalar.activation(rstd_row, var_row, AF.Sqrt, bias=eps_t, scale=1.0)
    with nc.allow_low_precision(reason="f32r rstd feeds PE broadcast matmul"):
        nc.vector.reciprocal(rstd_row, rstd_row)

    mu_b = ps_big.tile([128, L], FP32, tag="big")
    rstd_b = ps_big.tile([128, L], FP32, tag="big")
    for seg in range(2):
        sl = slice(seg * 512, (seg + 1) * 512)
        nc.tensor.matmul(mu_b[:, sl], ones_row, mu_row[:, sl])
        nc.tensor.matmul(rstd_b[:, sl], ones_row, rstd_row[:, sl])
    qnT = [sb.tile([128, L], F32R, tag=f"qnT{i}", name="qnT") for i in range(NDT)]
    for i in range(NDT):
        nc.vector.tensor_sub(qnT[i], qTt[i], mu_b)
        nc.vector.tensor_mul(qnT[i], qnT[i], rstd_b)
        nc.vector.tensor_scalar(qnT[i], qnT[i], g_col[i], b_col[i],
                                op0=OP.mult, op1=OP.add)

    # ---- Phase 2: projections ----
    # kh first (khT reuses qTt slots, dead after LN), then qh (reuses kTt
    # slots), then vh (outT later reuses vTt slots).
    khT = [sb.tile([128, L], F32R, tag=f"qT{i}", name="khT") for i in range(NET)]
    for et in range(NET):
        esl = slice(et * 128, (et + 1) * 128)
        for seg in range(2):
            sl = slice(seg * 512, (seg + 1) * 512)
            psk = ps_small.tile([128, 512], FP32, tag="small")
            for kt in range(NDT):
                nc.tensor.matmul(psk, wk[kt][:, esl], kTt[kt][:, sl],
                                 start=(kt == 0), stop=(kt == NDT - 1))
            nc.scalar.copy(khT[et][:, sl], psk)
    qhT = [sb.tile([128, L], F32R, tag=f"kT{i}", name="qhT") for i in range(NET)]
    for et in range(NET):
        esl = slice(et * 128, (et + 1) * 128)
        for seg in range(2):
            sl = slice(seg * 512, (seg + 1) * 512)
            psq = ps_small.tile([128, 512], FP32, tag="small")
            for kt in range(NDT):
                nc.tensor.matmul(psq, wq[kt][:, esl], qnT[kt][:, sl],
                                 start=(kt == 0), stop=(kt == NDT - 1))
            nc.scalar.copy(qhT[et][:, sl], psq)
    # V in natural layout [lk, e], bf16, with an appended ones column per head
    vh = [sb.tile([128, H, DV + 1], BF16, tag=f"vh{i}", name="vh") for i in range(NLT)]
    for lt in range(NLT):
        lsl = slice(lt * 128, (lt + 1) * 128)
        psv = ps_small.tile([128, 512], FP32, tag="small")
        for kt in range(NDT):
            nc.tensor.matmul(psv, vTt[kt][:, lsl], wv[kt],
                             start=(kt == 0), stop=(kt == NDT - 1))
        nc.scalar.copy(
            vh[lt][:, :, 0:DV], psv.rearrange("p (h d) -> p h d", h=H))
        nc.vector.memset(vh[lt][:, :, DV:DV + 1], 1.0)

    # ---- Phase 3: attention, one head at a time for deep pipelining ----
    # Consecutive heads alternate PE rows 0-63 / 64-127, so score matmuls of
    # head h+1 are row-disjoint from head h's and overlap on the PE array.
    # PV accumulation is interleaved into the lt loop so PE work tracks the
    # ACT exp chain instead of bunching at head end.
    outT = [sb.tile([128, L], F32R, tag=f"vT{i}", name="outT") for i in range(NET)]
    for h in range(H):
        et, sub = h // 2, h % 2
        esl = slice(64 * sub, 64 * sub + 64)
        psv = [ps_small.tile([128, 512], FP32, tag="small", name="psv")
               for _ in range(2)]
        for lt in range(NLT):
            lsl = slice(lt * 128, (lt + 1) * 128)
            pst = ps_big.tile([128, L], FP32, tag="big", name="pst")
            for seg in range(2):
                sl = slice(seg * 512, (seg + 1) * 512)
                nc.tensor.matmul(pst[:, sl], khT[et][esl, lsl], qhT[et][esl, sl])
            if et == 0:  # owned heads are always slots 0/1 (host permutes)
                stg = stage_pool.tile([128, L], FP32, tag="stg", name="stg")
                nc.vector.tensor_copy(stg, pst)
                nc.scalar.dma_start(out=attnT_d[sub, lt * 128:(lt + 1) * 128, :],
                                  in_=stg)
            e_t = et_pool.tile([128, L], BF16, tag="et", name="e_t")
            nc.scalar.activation(e_t, pst, AF.Exp)
            nc.vector.tensor_mul(e_t, e_t, maskT[lt])
            for seg in range(2):
                sl = slice(seg * 512, (seg + 1) * 512)
                nc.tensor.matmul(psv[seg][0:DV + 1, :], vh[lt][:, h, :],
                                 e_t[:, sl],
                                 start=(lt == 0), stop=(lt == NLT - 1),
                                 skip_group_check=True)
        den_t = sb.tile([1, L], F32R, tag="den", bufs=2, name="den")
        for seg in range(2):
            sl = slice(seg * 512, (seg + 1) * 512)
            nc.vector.tensor_copy(outT[et][esl, sl], psv[seg][0:DV, :])
            nc.vector.tensor_copy(den_t[:, sl], psv[seg][DV:DV + 1, :])
        # normalize this head: scale outT columns by 1/denom
        with nc.allow_low_precision(reason="f32r denom feeds PE broadcast matmul"):
            nc.vector.reciprocal(den_t, den_t)
        ps_s = [ps_small.tile([64, 512], FP32, tag="small", name="ps_s")
                for _ in range(2)]
        for seg in range(2):
            sl = slice(seg * 512, (seg + 1) * 512)
            nc.tensor.matmul(ps_s[seg], ones_row[0:1, 0:64], den_t[:, sl])
            nc.vector.tensor_mul(outT[et][esl, sl], outT[et][esl, sl], ps_s[seg])

    # ---- Phase 5: FC ----
    for lt in range(NLT):
        lsl = slice(lt * 128, (lt + 1) * 128)
        psf = ps_small.tile([128, 512], FP32, tag="small")
        for et in range(NET):
            nc.tensor.matmul(psf, outT[et][:, lsl], fcw[et],
                             start=(et == 0), stop=(et == NET - 1))
        fo = fco_pool.tile([128, D], FP32, tag="fo")
        nc.vector.tensor_copy(fo, psf)
        nc.scalar.dma_start(out=outu_d[lt * 128:(lt + 1) * 128, :], in_=fo)


def _build():
    global _COMPILED
    if _COMPILED is None:
        nc = bacc.Bacc("TRN2", target_bir_lowering=False, debug=False,
                       num_devices=NCORES)
        from contextlib import ExitStack
        with tile.TileContext(nc) as tc, ExitStack() as ctx:
            _emit(ctx, tc)
        nc.compile()
        _COMPILED = nc
    return _COMPILED


def kernel(q, k, v, mask, w_q, w_k, w_v, fc_w, fc_b, ln_g, ln_b):
    global LAST_EXEC_NS, LAST_RESULTS
    q = np.asarray(q, np.float32)
    k = np.asarray(k, np.float32)
    v = np.asarray(v, np.float32)
    mask = np.asarray(mask, np.float32)
    fc_b = np.asarray(fc_b, np.float32)
    ln_g = np.asarray(ln_g, np.float32)
    ln_b = np.asarray(ln_b, np.float32)
    wqT = np.ascontiguousarray(np.asarray(w_q, np.float32).T) / TEMP  # [D, H*DK]
    wkT = np.ascontiguousarray(np.asarray(w_k, np.float32).T)
    wvT = np.ascontiguousarray(np.asarray(w_v, np.float32).T)
    fcwT = np.ascontiguousarray(np.asarray(fc_w, np.float32).T)      # [H*DV, D]

    nc = _build()
    in_maps = []
    perms = []
    for c in range(NCORES):
        p, b = c // 2, c % 2
        perm = [2 * p, 2 * p + 1] + [h for h in range(H) if h not in (2 * p, 2 * p + 1)]
        perms.append(perm)
        wqT_p = np.ascontiguousarray(
            wqT.reshape(D, H, DK)[:, perm].reshape(D, D))
        wkT_p = np.ascontiguousarray(
            wkT.reshape(D, H, DK)[:, perm].reshape(D, D))
        wvT_p = np.ascontiguousarray(
            wvT.reshape(D, H, DV)[:, perm].reshape(D, D))
        fcwT_p = np.ascontiguousarray(
            fcwT.reshape(H, DV, D)[perm].reshape(D, D))
        in_maps.append({
            "qT": np.ascontiguousarray(q[b].T),
            "kT": np.ascontiguousarray(k[b].T),
            "vT": np.ascontiguousarray(v[b].T),
            "maskT": np.ascontiguousarray(mask[p, 0, 0].T).astype(ml_dtypes.bfloat16),
            "wqT": wqT_p, "wkT": wkT_p, "wvT": wvT_p, "fcwT": fcwT_p,
            "ln_g": np.ascontiguousarray(ln_g[:, None]),
            "ln_b": np.ascontiguousarray(ln_b[:, None]),
            "ones_c": np.ones((128, 1), np.float32),
            "ones_r": np.ones((1, 128), np.float32),
        })

    res = run_bass_kernel_spmd(nc, in_maps, core_ids=list(range(NCORES)),
                               trace=TRACE)
    LAST_EXEC_NS = res.exec_time_ns
    LAST_RESULTS = res

    out = np.empty((P, B, L, D), np.float32)
    attn = np.empty((B, H, L, L), np.float32)
    for c in range(NCORES):
        p, b = c // 2, c % 2
        rc = res.results[c]
        out[p, b] = rc["out_u"] + fc_b[None, :] + q[b]
        attn[b, 2 * p] = rc["attnT"][0].T
        attn[b, 2 * p + 1] = rc["attnT"][1].T
    return out, attn


# revision 20
# speedup vs baseline: 1.0143x; 1.0143x over previous
"""Trainium2 Bass kernel for MPMultiHeadAttention (P=4 masks, B=2, L=1024, D=512, H=8).

Sharding: 8 cores = (p, b) pairs; core c handles mask p=c//2, batch b=c%2.
Each core computes all H=8 heads for its (p,b):
  LN(q[b]) -> Q/K/V projections -> scores^T -> exp*mask -> PV (+denominator
  via an appended ones-column on V) -> normalize -> FC.
Per-core outputs: out_u = FC output [1024,512] (host adds fc_b + residual),
attnT = pre-softmax scores, transposed, for 2 heads [2,1024,1024] (host
transposes back). Head ownership is made SPMD-uniform by permuting heads
host-side so each core's owned heads are head-slots 0 and 1.

All activations live in transposed layout (feature dim on partitions) so every
matmul contraction runs without on-device transposes; host pre-transposes
q/k/v/mask and the weights (pure layout prep on the sharded inputs).
"""

import numpy as np
import ml_dtypes

import concourse.bacc as bacc
import concourse.bass as bass
import concourse.tile as tile
from concourse import mybir
from concourse.bass_utils import run_bass_kernel_spmd

B, L, D = 2, 1024, 512
H, DK, DV, P = 8, 64, 64, 4
TEMP = float(DK) ** 0.5
EPS = 1e-6
NCORES = 8

FP32 = mybir.dt.float32
F32R = mybir.dt.float32r
BF16 = mybir.dt.bfloat16

NLT = L // 128          # 8 lq/lk tiles of 128
NDT = D // 128          # 4 d tiles of 128
NET = D // 128          # 4 e tiles (H*DK = 512)

# Profiling knobs (test.py sets TRACE=True; harness leaves it False)
TRACE = False
LAST_EXEC_NS = None
LAST_RESULTS = None

_COMPILED = None


def _emit(ctx, tc):
    nc = tc.nc
    AF = mybir.ActivationFunctionType
    OP = mybir.AluOpType

    # ---- DRAM I/O ----
    qT_d = nc.dram_tensor("qT", [D, L], F32R, kind="ExternalInput").ap()
    kT_d = nc.dram_tensor("kT", [D, L], F32R, kind="ExternalInput").ap()
    vT_d = nc.dram_tensor("vT", [D, L], F32R, kind="ExternalInput").ap()
    maskT_d = nc.dram_tensor("maskT", [L, L], BF16, kind="ExternalInput").ap()
    wqT_d = nc.dram_tensor("wqT", [D, D], F32R, kind="ExternalInput").ap()
    wkT_d = nc.dram_tensor("wkT", [D, D], F32R, kind="ExternalInput").ap()
    wvT_d = nc.dram_tensor("wvT", [D, D], F32R, kind="ExternalInput").ap()
    fcwT_d = nc.dram_tensor("fcwT", [D, D], F32R, kind="ExternalInput").ap()
    lng_d = nc.dram_tensor("ln_g", [D, 1], FP32, kind="ExternalInput").ap()
    lnb_d = nc.dram_tensor("ln_b", [D, 1], FP32, kind="ExternalInput").ap()
    onesc_d = nc.dram_tensor("ones_c", [128, 1], F32R, kind="ExternalInput").ap()
    onesr_d = nc.dram_tensor("ones_r", [1, 128], F32R, kind="ExternalInput").ap()
    outu_d = nc.dram_tensor("out_u", [L, D], FP32, kind="ExternalOutput").ap()
    attnT_d = nc.dram_tensor("attnT", [2, L, L], FP32, kind="ExternalOutput").ap()

    const = ctx.enter_context(tc.tile_pool(name="const", bufs=1))
    sb = ctx.enter_context(tc.tile_pool(name="sb", bufs=1))
    sq_pool = ctx.enter_context(tc.tile_pool(name="sqp", bufs=4))
    et_pool = ctx.enter_context(tc.tile_pool(name="etp", bufs=16))
    stage_pool = ctx.enter_context(tc.tile_pool(name="stp", bufs=2))
    fco_pool = ctx.enter_context(tc.tile_pool(name="fco", bufs=2))
    ps_big = ctx.enter_context(
        tc.tile_pool(name="psb", bufs=2, space=bass.MemorySpace.PSUM))
    ps_small = ctx.enter_context(
        tc.tile_pool(name="pss", bufs=4, space=bass.MemorySpace.PSUM))

    # ---- constants / weights ----
    wq = [const.tile([128, D], F32R, tag=f"wq{i}", name="wq") for i in range(NDT)]
    wk = [const.tile([128, D], F32R, tag=f"wk{i}", name="wk") for i in range(NDT)]
    wv = [const.tile([128, D], F32R, tag=f"wv{i}", name="wv") for i in range(NDT)]
    fcw = [const.tile([128, D], F32R, tag=f"fcw{i}", name="fcw") for i in range(NET)]
    for i in range(NDT):
        nc.sync.dma_start(out=wq[i], in_=wqT_d[i * 128:(i + 1) * 128, :])
        nc.sync.dma_start(out=wk[i], in_=wkT_d[i * 128:(i + 1) * 128, :])
        nc.sync.dma_start(out=wv[i], in_=wvT_d[i * 128:(i + 1) * 128, :])
        nc.sync.dma_start(out=fcw[i], in_=fcwT_d[i * 128:(i + 1) * 128, :])
    g_col = [const.tile([128, 1], FP32, tag=f"gc{i}", name="gc") for i in range(NDT)]
    b_col = [const.tile([128, 1], FP32, tag=f"bc{i}", name="bc") for i in range(NDT)]
    for i in range(NDT):
        nc.sync.dma_start(out=g_col[i], in_=lng_d[i * 128:(i + 1) * 128, :])
        nc.sync.dma_start(out=b_col[i], in_=lnb_d[i * 128:(i + 1) * 128, :])
    ones_col = const.tile([128, 1], F32R, tag="oc")
    nc.sync.dma_start(out=ones_col, in_=onesc_d)
    ones_row = const.tile([1, 128], F32R, tag="orow")
    nc.sync.dma_start(out=ones_row, in_=onesr_d)
    eps_t = const.tile([1, 1], FP32, tag="eps")
    nc.vector.memset(eps_t, EPS)

    # ---- activations ----
    qTt = [sb.tile([128, L], F32R, tag=f"qT{i}", name="qTt") for i in range(NDT)]
    kTt = [sb.tile([128, L], F32R, tag=f"kT{i}", name="kTt") for i in range(NDT)]
    vTt = [sb.tile([128, L], F32R, tag=f"vT{i}", name="vTt") for i in range(NDT)]
    for i in range(NDT):
        for sg in range(2):
            csl = slice(sg * 512, (sg + 1) * 512)
            nc.sync.dma_start(out=qTt[i][:, csl],
                              in_=qT_d[i * 128:(i + 1) * 128, csl])
            nc.sync.dma_start(out=kTt[i][:, csl],
                              in_=kT_d[i * 128:(i + 1) * 128, csl])
            nc.sync.dma_start(out=vTt[i][:, csl],
                              in_=vT_d[i * 128:(i + 1) * 128, csl])
    maskT = [sb.tile([128, L], BF16, tag=f"mask{j}", name="maskT") for j in range(NLT)]
    for j in range(NLT):
        nc.sync.dma_start(out=maskT[j], in_=maskT_d[j * 128:(j + 1) * 128, :])

    # ---- Phase 1: LayerNorm of q, in transposed layout ----
    # col sums via ones-matmuls; mean/var rows; broadcast rows back via matmul
    mu_row = sb.tile([1, L], F32R, tag="mu")
    var_row = sb.tile([1, L], FP32, tag="var")
    rstd_row = sb.tile([1, L], F32R, tag="rstd")
    sqs = [sq_pool.tile([128, L], F32R, tag="sq", name="sq") for _ in range(NDT)]
    for i in range(NDT):
        for sg in range(2):
            csl = slice(sg * 512, (sg + 1) * 512)
            nc.vector.tensor_mul(sqs[i][:, csl], qTt[i][:, csl],
                                 qTt[i][:, csl])
    for seg in range(2):
        sl = slice(seg * 512, (seg + 1) * 512)
        ps1 = ps_small.tile([1, 512], FP32, tag="small")
        for i in range(NDT):
            nc.tensor.matmul(ps1, ones_col, qTt[i][:, sl],
                             start=(i == 0), stop=(i == NDT - 1))
        nc.scalar.mul(mu_row[:, sl], ps1, 1.0 / D)
        ps2 = ps_small.tile([1, 512], FP32, tag="small")
        for i in range(NDT):
            nc.tensor.matmul(ps2, ones_col, sqs[i][:, sl],
                             start=(i == 0), stop=(i == NDT - 1))
        nc.scalar.mul(var_row[:, sl], ps2, 1.0 / D)
    # var = E[x^2] - mu^2 ; rstd = 1/sqrt(var+eps)
    nc.vector.tensor_mul(rstd_row, mu_row, mu_row)       # rstd_row = mu^2 (tmp)
    nc.vector.tensor_sub(var_row, var_row, rstd_row)
    nc.scalar.activation(rstd_row, var_row, AF.Sqrt, bias=eps_t, scale=1.0)
    with nc.allow_low_precision(reason="f32r rstd feeds PE broadcast matmul"):
        nc.vector.reciprocal(rstd_row, rstd_row)

    mu_b = ps_big.tile([128, L], FP32, tag="big")
    rstd_b = ps_big.tile([128, L], FP32, tag="big")
    for seg in range(2):
        sl = slice(seg * 512, (seg + 1) * 512)
        nc.tensor.matmul(mu_b[:, sl], ones_row, mu_row[:, sl])
        nc.tensor.matmul(rstd_b[:, sl], ones_row, rstd_row[:, sl])
    qnT = [sb.tile([128, L], F32R, tag=f"qnT{i}", name="qnT") for i in range(NDT)]
    for i in range(NDT):
        nc.vector.tensor_sub(qnT[i], qTt[i], mu_b)
        nc.vector.tensor_mul(qnT[i], qnT[i], rstd_b)
        nc.vector.tensor_scalar(qnT[i], qnT[i], g_col[i], b_col[i],
                                op0=OP.mult, op1=OP.add)

    # ---- Phase 2: projections ----
    # kh first (khT reuses qTt slots, dead after LN), then qh (reuses kTt
    # slots), then vh (outT later reuses vTt slots).
    khT = [sb.tile([128, L], F32R, tag=f"qT{i}", name="khT") for i in range(NET)]
    for et in range(NET):
        esl = slice(et * 128, (et + 1) * 128)
        for seg in range(2):
            sl = slice(seg * 512, (seg + 1) * 512)
            psk = ps_small.tile([128, 512], FP32, tag="small")
            for kt in range(NDT):
                nc.tensor.matmul(psk, wk[kt][:, esl], kTt[kt][:, sl],
                                 start=(kt == 0), stop=(kt == NDT - 1))
            nc.scalar.copy(khT[et][:, sl], psk)
    qhT = [sb.tile([128, L], F32R, tag=f"kT{i}", name="qhT") for i in range(NET)]
    for et in range(NET):
        esl = slice(et * 128, (et + 1) * 128)
        for seg in range(2):
            sl = slice(seg * 512, (seg + 1) * 512)
            psq = ps_small.tile([128, 512], FP32, tag="small")
            for kt in range(NDT):
                nc.tensor.matmul(psq, wq[kt][:, esl], qnT[kt][:, sl],
                                 start=(kt == 0), stop=(kt == NDT - 1))
            nc.scalar.copy(qhT[et][:, sl], psq)
    # V in natural layout [lk, e], bf16, with an appended ones column per head
    vh = [sb.tile([128, H, DV + 1], BF16, tag=f"vh{i}", name="vh") for i in range(NLT)]
    for lt in range(NLT):
        lsl = slice(lt * 128, (lt + 1) * 128)
        psv = ps_small.tile([128, 512], FP32, tag="small")
        for kt in range(NDT):
            nc.tensor.matmul(psv, vTt[kt][:, lsl], wv[kt],
                             start=(kt == 0), stop=(kt == NDT - 1))
        nc.scalar.copy(
            vh[lt][:, :, 0:DV], psv.rearrange("p (h d) -> p h d", h=H))
        nc.vector.memset(vh[lt][:, :, DV:DV + 1], 1.0)

    # ---- Phase 3: attention, one head at a time for deep pipelining ----
    # Consecutive heads alternate PE rows 0-63 / 64-127, so score matmuls of
    # head h+1 are row-disjoint from head h's and overlap on the PE array.
    # PV accumulation is interleaved into the lt loop so PE work tracks the
    # ACT exp chain instead of bunching at head end.
    outT = [sb.tile([128, L], F32R, tag=f"vT{i}", name="outT") for i in range(NET)]
    for h in range(H):
        et, sub = h // 2, h % 2
        esl = slice(64 * sub, 64 * sub + 64)
        psv = [ps_small.tile([128, 512], FP32, tag="small", name="psv")
               for _ in range(2)]
        for lt in range(NLT):
            lsl = slice(lt * 128, (lt + 1) * 128)
            pst = ps_big.tile([128, L], FP32, tag="big", name="pst")
            for seg in range(2):
                sl = slice(seg * 512, (seg + 1) * 512)
                nc.tensor.matmul(pst[:, sl], khT[et][esl, lsl], qhT[et][esl, sl])
            if et == 0:  # owned heads are always slots 0/1 (host permutes)
                stg = stage_pool.tile([128, L], FP32, tag="stg", name="stg")
                nc.vector.tensor_copy(stg, pst)
                nc.sync.dma_start(out=attnT_d[sub, lt * 128:(lt + 1) * 128, :],
                                  in_=stg)
            e_t = et_pool.tile([128, L], BF16, tag="et", name="e_t")
            nc.scalar.activation(e_t, pst, AF.Exp)
            nc.vector.tensor_mul(e_t, e_t, maskT[lt])
            for seg in range(2):
                sl = slice(seg * 512, (seg + 1) * 512)
                nc.tensor.matmul(psv[seg][0:DV + 1, :], vh[lt][:, h, :],
                                 e_t[:, sl],
                                 start=(lt == 0), stop=(lt == NLT - 1),
                                 skip_group_check=True)
        den_t = sb.tile([1, L], F32R, tag="den", bufs=2, name="den")
        for seg in range(2):
            sl = slice(seg * 512, (seg + 1) * 512)
            nc.vector.tensor_copy(outT[et][esl, sl], psv[seg][0:DV, :])
            nc.vector.tensor_copy(den_t[:, sl], psv[seg][DV:DV + 1, :])
        # normalize this head: scale outT columns by 1/denom
        with nc.allow_low_precision(reason="f32r denom feeds PE broadcast matmul"):
            nc.vector.reciprocal(den_t, den_t)
        ps_s = [ps_small.tile([64, 512], FP32, tag="small", name="ps_s")
                for _ in range(2)]
        for seg in range(2):
            sl = slice(seg * 512, (seg + 1) * 512)
            nc.tensor.matmul(ps_s[seg], ones_row[0:1, 0:64], den_t[:, sl])
            nc.vector.tensor_mul(outT[et][esl, sl], outT[et][esl, sl], ps_s[seg])

    # ---- Phase 5: FC ----
    for lt in range(NLT):
        lsl = slice(lt * 128, (lt + 1) * 128)
        psf = ps_small.tile([128, 512], FP32, tag="small")
        for et in range(NET):
            nc.tensor.matmul(psf, outT[et][:, lsl], fcw[et],
                             start=(et == 0), stop=(et == NET - 1))
        fo = fco_pool.tile([128, D], FP32, tag="fo")
        nc.vector.tensor_copy(fo, psf)
        nc.scalar.dma_start(out=outu_d[lt * 128:(lt + 1) * 128, :], in_=fo)


def _build():
    global _COMPILED
    if _COMPILED is None:
        nc = bacc.Bacc("TRN2", target_bir_lowering=False, debug=False,
                       num_devices=NCORES)
        from contextlib import ExitStack
        with tile.TileContext(nc) as tc, ExitStack() as ctx:
            _emit(ctx, tc)
        nc.compile()
        _COMPILED = nc
    return _COMPILED


def kernel(q, k, v, mask, w_q, w_k, w_v, fc_w, fc_b, ln_g, ln_b):
    global LAST_EXEC_NS, LAST_RESULTS
    q = np.asarray(q, np.float32)
    k = np.asarray(k, np.float32)
    v = np.asarray(v, np.float32)
    mask = np.asarray(mask, np.float32)
    fc_b = np.asarray(fc_b, np.float32)
    ln_g = np.asarray(ln_g, np.float32)
    ln_b = np.asarray(ln_b, np.float32)
    wqT = np.ascontiguousarray(np.asarray(w_q, np.float32).T) / TEMP  # [D, H*DK]
    wkT = np.ascontiguousarray(np.asarray(w_k, np.float32).T)
    wvT = np.ascontiguousarray(np.asarray(w_v, np.float32).T)
    fcwT = np.ascontiguousarray(np.asarray(fc_w, np.float32).T)      # [H*DV, D]

    nc = _build()
    in_maps = []
    perms = []
    for c in range(NCORES):
        p, b = c // 2, c % 2
        perm = [2 * p, 2 * p + 1] + [h for h in range(H) if h not in (2 * p, 2 * p + 1)]
        perms.append(perm)
        wqT_p = np.ascontiguousarray(
            wqT.reshape(D, H, DK)[:, perm].reshape(D, D))
        wkT_p = np.ascontiguousarray(
            wkT.reshape(D, H, DK)[:, perm].reshape(D, D))
        wvT_p = np.ascontiguousarray(
            wvT.reshape(D, H, DV)[:, perm].reshape(D, D))
        fcwT_p = np.ascontiguousarray(
            fcwT.reshape(H, DV, D)[perm].reshape(D, D))
        in_maps.append({
            "qT": np.ascontiguousarray(q[b].T),
            "kT": np.ascontiguousarray(k[b].T),
            "vT": np.ascontiguousarray(v[b].T),
            "maskT": np.ascontiguousarray(mask[p, 0, 0].T).astype(ml_dtypes.bfloat16),
            "wqT": wqT_p, "wkT": wkT_p, "wvT": wvT_p, "fcwT": fcwT_p,
            "ln_g": np.ascontiguousarray(ln_g[:, None]),
            "ln_b": np.ascontiguousarray(ln_b[:, None]),
            "ones_c": np.ones((128, 1), np.float32),
            "ones_r": np.ones((1, 128), np.float32),
        })

    res = run_bass_kernel_spmd(nc, in_maps, core_ids=list(range(NCORES)),
                               trace=TRACE)
    LAST_EXEC_NS = res.exec_time_ns
    LAST_RESULTS = res

    out = np.empty((P, B, L, D), np.float32)
    attn = np.empty((B, H, L, L), np.float32)
    for c in range(NCORES):
        p, b = c // 2, c % 2
        rc = res.results[c]
        out[p, b] = rc["out_u"] + fc_b[None, :] + q[b]
        attn[b, 2 * p] = rc["attnT"][0].T
        attn[b, 2 * p + 1] = rc["attnT"][1].T
    return out, attn


# revision 21
# speedup vs baseline: 1.0907x; 1.0753x over previous
"""Trainium2 Bass kernel for MPMultiHeadAttention (P=4 masks, B=2, L=1024, D=512, H=8).

Sharding: 8 cores = (p, b) pairs; core c handles mask p=c//2, batch b=c%2.
Each core computes all H=8 heads for its (p,b):
  LN(q[b]) -> Q/K/V projections -> scores^T -> exp*mask -> PV (+denominator
  via an appended ones-column on V) -> normalize -> FC.
Per-core outputs: out_u = FC output [1024,512] (host adds fc_b + residual),
attnT = pre-softmax scores, transposed, for 2 heads [2,1024,1024] (host
transposes back). Head ownership is made SPMD-uniform by permuting heads
host-side so each core's owned heads are head-slots 0 and 1.

All activations live in transposed layout (feature dim on partitions) so every
matmul contraction runs without on-device transposes; host pre-transposes
q/k/v/mask and the weights (pure layout prep on the sharded inputs).
"""

import numpy as np
import ml_dtypes

import concourse.bacc as bacc
import concourse.bass as bass
import concourse.tile as tile
from concourse import mybir
from concourse.bass_utils import run_bass_kernel_spmd

B, L, D = 2, 1024, 512
H, DK, DV, P = 8, 64, 64, 4
TEMP = float(DK) ** 0.5
EPS = 1e-6
NCORES = 8

FP32 = mybir.dt.float32
F32R = mybir.dt.float32r
BF16 = mybir.dt.bfloat16

NLT = L // 128          # 8 lq/lk tiles of 128
NDT = D // 128          # 4 d tiles of 128
NET = D // 128          # 4 e tiles (H*DK = 512)

# Profiling knobs (test.py sets TRACE=True; harness leaves it False)
TRACE = False
LAST_EXEC_NS = None
LAST_RESULTS = None

_COMPILED = None


def _emit(ctx, tc):
    nc = tc.nc
    AF = mybir.ActivationFunctionType
    OP = mybir.AluOpType

    # ---- DRAM I/O ----
    qT_d = nc.dram_tensor("qT", [D, L], F32R, kind="ExternalInput").ap()
    kT_d = nc.dram_tensor("kT", [D, L], F32R, kind="ExternalInput").ap()
    vT_d = nc.dram_tensor("vT", [D, L], F32R, kind="ExternalInput").ap()
    maskT_d = nc.dram_tensor("maskT", [L, L], BF16, kind="ExternalInput").ap()
    wqT_d = nc.dram_tensor("wqT", [D, D], F32R, kind="ExternalInput").ap()
    wkT_d = nc.dram_tensor("wkT", [D, D], F32R, kind="ExternalInput").ap()
    wvT_d = nc.dram_tensor("wvT", [D, D], F32R, kind="ExternalInput").ap()
    fcwT_d = nc.dram_tensor("fcwT", [D, D], F32R, kind="ExternalInput").ap()
    lng_d = nc.dram_tensor("ln_g", [D, 1], FP32, kind="ExternalInput").ap()
    lnb_d = nc.dram_tensor("ln_b", [D, 1], FP32, kind="ExternalInput").ap()
    onesc_d = nc.dram_tensor("ones_c", [128, 1], F32R, kind="ExternalInput").ap()
    onesr_d = nc.dram_tensor("ones_r", [1, 128], F32R, kind="ExternalInput").ap()
    outu_d = nc.dram_tensor("out_u", [L, D], FP32, kind="ExternalOutput").ap()
    attnT_d = nc.dram_tensor("attnT", [2, L, L], FP32, kind="ExternalOutput").ap()

    const = ctx.enter_context(tc.tile_pool(name="const", bufs=1))
    sb = ctx.enter_context(tc.tile_pool(name="sb", bufs=1))
    sq_pool = ctx.enter_context(tc.tile_pool(name="sqp", bufs=4))
    et_pool = ctx.enter_context(tc.tile_pool(name="etp", bufs=16))
    stage_pool = ctx.enter_context(tc.tile_pool(name="stp", bufs=2))
    fco_pool = ctx.enter_context(tc.tile_pool(name="fco", bufs=2))
    ps_big = ctx.enter_context(
        tc.tile_pool(name="psb", bufs=2, space=bass.MemorySpace.PSUM))
    ps_small = ctx.enter_context(
        tc.tile_pool(name="pss", bufs=4, space=bass.MemorySpace.PSUM))

    # ---- constants / weights ----
    wq = [const.tile([128, D], F32R, tag=f"wq{i}", name="wq") for i in range(NDT)]
    wk = [const.tile([128, D], F32R, tag=f"wk{i}", name="wk") for i in range(NDT)]
    wv = [const.tile([128, D], F32R, tag=f"wv{i}", name="wv") for i in range(NDT)]
    fcw = [const.tile([128, D], F32R, tag=f"fcw{i}", name="fcw") for i in range(NET)]
    g_col = [const.tile([128, 1], FP32, tag=f"gc{i}", name="gc") for i in range(NDT)]
    b_col = [const.tile([128, 1], FP32, tag=f"bc{i}", name="bc") for i in range(NDT)]
    ones_col = const.tile([128, 1], F32R, tag="oc")
    nc.sync.dma_start(out=ones_col, in_=onesc_d)
    ones_row = const.tile([1, 128], F32R, tag="orow")
    nc.sync.dma_start(out=ones_row, in_=onesr_d)
    eps_t = const.tile([1, 1], FP32, tag="eps")
    nc.vector.memset(eps_t, EPS)

    # ---- activations ----
    qTt = [sb.tile([128, L], F32R, tag=f"qT{i}", name="qTt") for i in range(NDT)]
    kTt = [sb.tile([128, L], F32R, tag=f"kT{i}", name="kTt") for i in range(NDT)]
    vTt = [sb.tile([128, L], F32R, tag=f"vT{i}", name="vTt") for i in range(NDT)]
    for i in range(NDT):
        for sg in range(2):
            csl = slice(sg * 512, (sg + 1) * 512)
            nc.sync.dma_start(out=qTt[i][:, csl],
                              in_=qT_d[i * 128:(i + 1) * 128, csl])
            nc.sync.dma_start(out=kTt[i][:, csl],
                              in_=kT_d[i * 128:(i + 1) * 128, csl])
            nc.sync.dma_start(out=vTt[i][:, csl],
                              in_=vT_d[i * 128:(i + 1) * 128, csl])
    maskT = [sb.tile([128, L], BF16, tag=f"mask{j}", name="maskT") for j in range(NLT)]
    for i in range(NDT):
        nc.sync.dma_start(out=g_col[i], in_=lng_d[i * 128:(i + 1) * 128, :])
        nc.sync.dma_start(out=b_col[i], in_=lnb_d[i * 128:(i + 1) * 128, :])
    for i in range(NDT):
        nc.sync.dma_start(out=wk[i], in_=wkT_d[i * 128:(i + 1) * 128, :])
        nc.sync.dma_start(out=wq[i], in_=wqT_d[i * 128:(i + 1) * 128, :])
        nc.sync.dma_start(out=wv[i], in_=wvT_d[i * 128:(i + 1) * 128, :])
    for j in range(NLT):
        nc.sync.dma_start(out=maskT[j], in_=maskT_d[j * 128:(j + 1) * 128, :])
    for i in range(NDT):
        nc.sync.dma_start(out=fcw[i], in_=fcwT_d[i * 128:(i + 1) * 128, :])

    # ---- Phase 1: LayerNorm of q, in transposed layout ----
    # col sums via ones-matmuls; mean/var rows; broadcast rows back via matmul
    mu_row = sb.tile([1, L], F32R, tag="mu")
    var_row = sb.tile([1, L], FP32, tag="var")
    rstd_row = sb.tile([1, L], F32R, tag="rstd")
    sqs = [sq_pool.tile([128, L], F32R, tag="sq", name="sq") for _ in range(NDT)]
    for i in range(NDT):
        for sg in range(2):
            csl = slice(sg * 512, (sg + 1) * 512)
            nc.vector.tensor_mul(sqs[i][:, csl], qTt[i][:, csl],
                                 qTt[i][:, csl])
    for seg in range(2):
        sl = slice(seg * 512, (seg + 1) * 512)
        ps1 = ps_small.tile([1, 512], FP32, tag="small")
        for i in range(NDT):
            nc.tensor.matmul(ps1, ones_col, qTt[i][:, sl],
                             start=(i == 0), stop=(i == NDT - 1))
        nc.scalar.mul(mu_row[:, sl], ps1, 1.0 / D)
        ps2 = ps_small.tile([1, 512], FP32, tag="small")
        for i in range(NDT):
            nc.tensor.matmul(ps2, ones_col, sqs[i][:, sl],
                             start=(i == 0), stop=(i == NDT - 1))
        nc.scalar.mul(var_row[:, sl], ps2, 1.0 / D)
    # var = E[x^2] - mu^2 ; rstd = 1/sqrt(var+eps)
    nc.vector.tensor_mul(rstd_row, mu_row, mu_row)       # rstd_row = mu^2 (tmp)
    nc.vector.tensor_sub(var_row, var_row, rstd_row)
    nc.scalar.activation(rstd_row, var_row, AF.Sqrt, bias=eps_t, scale=1.0)
    with nc.allow_low_precision(reason="f32r rstd feeds PE broadcast matmul"):
        nc.vector.reciprocal(rstd_row, rstd_row)

    mu_b = ps_big.tile([128, L], FP32, tag="big")
    rstd_b = ps_big.tile([128, L], FP32, tag="big")
    for seg in range(2):
        sl = slice(seg * 512, (seg + 1) * 512)
        nc.tensor.matmul(mu_b[:, sl], ones_row, mu_row[:, sl])
        nc.tensor.matmul(rstd_b[:, sl], ones_row, rstd_row[:, sl])
    qnT = [sb.tile([128, L], F32R, tag=f"qnT{i}", name="qnT") for i in range(NDT)]
    for i in range(NDT):
        nc.vector.tensor_sub(qnT[i], qTt[i], mu_b)
        nc.vector.tensor_mul(qnT[i], qnT[i], rstd_b)
        nc.vector.tensor_scalar(qnT[i], qnT[i], g_col[i], b_col[i],
                                op0=OP.mult, op1=OP.add)

    # ---- Phase 2: projections ----
    # kh first (khT reuses qTt slots, dead after LN), then qh (reuses kTt
    # slots), then vh (outT later reuses vTt slots).
    khT = [sb.tile([128, L], F32R, tag=f"qT{i}", name="khT") for i in range(NET)]
    for et in range(NET):
        esl = slice(et * 128, (et + 1) * 128)
        for seg in range(2):
            sl = slice(seg * 512, (seg + 1) * 512)
            psk = ps_small.tile([128, 512], FP32, tag="small")
            for kt in range(NDT):
                nc.tensor.matmul(psk, wk[kt][:, esl], kTt[kt][:, sl],
                                 start=(kt == 0), stop=(kt == NDT - 1))
            nc.scalar.copy(khT[et][:, sl], psk)
    qhT = [sb.tile([128, L], F32R, tag=f"kT{i}", name="qhT") for i in range(NET)]
    for et in range(NET):
        esl = slice(et * 128, (et + 1) * 128)
        for seg in range(2):
            sl = slice(seg * 512, (seg + 1) * 512)
            psq = ps_small.tile([128, 512], FP32, tag="small")
            for kt in range(NDT):
                nc.tensor.matmul(psq, wq[kt][:, esl], qnT[kt][:, sl],
                                 start=(kt == 0), stop=(kt == NDT - 1))
            nc.scalar.copy(qhT[et][:, sl], psq)
    # V in natural layout [lk, e], bf16, with an appended ones column per head
    vh = [sb.tile([128, H, DV + 1], BF16, tag=f"vh{i}", name="vh") for i in range(NLT)]
    for lt in range(NLT):
        lsl = slice(lt * 128, (lt + 1) * 128)
        psv = ps_small.tile([128, 512], FP32, tag="small")
        for kt in range(NDT):
            nc.tensor.matmul(psv, vTt[kt][:, lsl], wv[kt],
                             start=(kt == 0), stop=(kt == NDT - 1))
        nc.scalar.copy(
            vh[lt][:, :, 0:DV], psv.rearrange("p (h d) -> p h d", h=H))
        nc.vector.memset(vh[lt][:, :, DV:DV + 1], 1.0)

    # ---- Phase 3: attention, one head at a time for deep pipelining ----
    # Consecutive heads alternate PE rows 0-63 / 64-127, so score matmuls of
    # head h+1 are row-disjoint from head h's and overlap on the PE array.
    # PV accumulation is interleaved into the lt loop so PE work tracks the
    # ACT exp chain instead of bunching at head end.
    outT = [sb.tile([128, L], F32R, tag=f"vT{i}", name="outT") for i in range(NET)]
    for h in range(H):
        et, sub = h // 2, h % 2
        esl = slice(64 * sub, 64 * sub + 64)
        psv = [ps_small.tile([128, 512], FP32, tag="small", name="psv")
               for _ in range(2)]
        for lt in range(NLT):
            lsl = slice(lt * 128, (lt + 1) * 128)
            pst = ps_big.tile([128, L], FP32, tag="big", name="pst")
            for seg in range(2):
                sl = slice(seg * 512, (seg + 1) * 512)
                nc.tensor.matmul(pst[:, sl], khT[et][esl, lsl], qhT[et][esl, sl])
            if et == 0:  # owned heads are always slots 0/1 (host permutes)
                stg = stage_pool.tile([128, L], FP32, tag="stg", name="stg")
                nc.vector.tensor_copy(stg, pst)
                nc.sync.dma_start(out=attnT_d[sub, lt * 128:(lt + 1) * 128, :],
                                  in_=stg)
            e_t = et_pool.tile([128, L], BF16, tag="et", name="e_t")
            nc.scalar.activation(e_t, pst, AF.Exp)
            nc.vector.tensor_mul(e_t, e_t, maskT[lt])
            for seg in range(2):
                sl = slice(seg * 512, (seg + 1) * 512)
                nc.tensor.matmul(psv[seg][0:DV + 1, :], vh[lt][:, h, :],
                                 e_t[:, sl],
                                 start=(lt == 0), stop=(lt == NLT - 1),
                                 skip_group_check=True)
        den_t = sb.tile([1, L], F32R, tag="den", bufs=2, name="den")
        for seg in range(2):
            sl = slice(seg * 512, (seg + 1) * 512)
            nc.vector.tensor_copy(outT[et][esl, sl], psv[seg][0:DV, :])
            nc.vector.tensor_copy(den_t[:, sl], psv[seg][DV:DV + 1, :])
        # normalize this head: scale outT columns by 1/denom
        with nc.allow_low_precision(reason="f32r denom feeds PE broadcast matmul"):
            nc.vector.reciprocal(den_t, den_t)
        ps_s = [ps_small.tile([64, 512], FP32, tag="small", name="ps_s")
                for _ in range(2)]
        for seg in range(2):
            sl = slice(seg * 512, (seg + 1) * 512)
            nc.tensor.matmul(ps_s[seg], ones_row[0:1, 0:64], den_t[:, sl])
            nc.vector.tensor_mul(outT[et][esl, sl], outT[et][esl, sl], ps_s[seg])

    # ---- Phase 5: FC ----
    for lt in range(NLT):
        lsl = slice(lt * 128, (lt + 1) * 128)
        psf = ps_small.tile([128, 512], FP32, tag="small")
        for et in range(NET):
            nc.tensor.matmul(psf, outT[et][:, lsl], fcw[et],
                             start=(et == 0), stop=(et == NET - 1))
        fo = fco_pool.tile([128, D], FP32, tag="fo")
        nc.vector.tensor_copy(fo, psf)
        nc.scalar.dma_start(out=outu_d[lt * 128:(lt + 1) * 128, :], in_=fo)


def _build():
    global _COMPILED
    if _COMPILED is None:
        nc = bacc.Bacc("TRN2", target_bir_lowering=False, debug=False,
                       num_devices=NCORES)
        from contextlib import ExitStack
        with tile.TileContext(nc) as tc, ExitStack() as ctx:
            _emit(ctx, tc)
        nc.compile()
        _COMPILED = nc
    return _COMPILED


def kernel(q, k, v, mask, w_q, w_k, w_v, fc_w, fc_b, ln_g, ln_b):
    global LAST_EXEC_NS, LAST_RESULTS
    q = np.asarray(q, np.float32)
    k = np.asarray(k, np.float32)
    v = np.asarray(v, np.float32)
    mask = np.asarray(mask, np.float32)
    fc_b = np.asarray(fc_b, np.float32)
    ln_g = np.asarray(ln_g, np.float32)
    ln_b = np.asarray(ln_b, np.float32)
    wqT = np.ascontiguousarray(np.asarray(w_q, np.float32).T) / TEMP  # [D, H*DK]
    wkT = np.ascontiguousarray(np.asarray(w_k, np.float32).T)
    wvT = np.ascontiguousarray(np.asarray(w_v, np.float32).T)
    fcwT = np.ascontiguousarray(np.asarray(fc_w, np.float32).T)      # [H*DV, D]

    nc = _build()
    in_maps = []
    perms = []
    for c in range(NCORES):
        p, b = c // 2, c % 2
        perm = [2 * p, 2 * p + 1] + [h for h in range(H) if h not in (2 * p, 2 * p + 1)]
        perms.append(perm)
        wqT_p = np.ascontiguousarray(
            wqT.reshape(D, H, DK)[:, perm].reshape(D, D))
        wkT_p = np.ascontiguousarray(
            wkT.reshape(D, H, DK)[:, perm].reshape(D, D))
        wvT_p = np.ascontiguousarray(
            wvT.reshape(D, H, DV)[:, perm].reshape(D, D))
        fcwT_p = np.ascontiguousarray(
            fcwT.reshape(H, DV, D)[perm].reshape(D, D))
        in_maps.append({
            "qT": np.ascontiguousarray(q[b].T),
            "kT": np.ascontiguousarray(k[b].T),
            "vT": np.ascontiguousarray(v[b].T),
            "maskT": np.ascontiguousarray(mask[p, 0, 0].T).astype(ml_dtypes.bfloat16),
            "wqT": wqT_p, "wkT": wkT_p, "wvT": wvT_p, "fcwT": fcwT_p,
            "ln_g": np.ascontiguousarray(ln_g[:, None]),
            "ln_b": np.ascontiguousarray(ln_b[:, None]),
            "ones_c": np.ones((128, 1), np.float32),
            "ones_r": np.ones((1, 128), np.float32),
        })

    res = run_bass_kernel_spmd(nc, in_maps, core_ids=list(range(NCORES)),
                               trace=TRACE)
    LAST_EXEC_NS = res.exec_time_ns
    LAST_RESULTS = res

    out = np.empty((P, B, L, D), np.float32)
    attn = np.empty((B, H, L, L), np.float32)
    for c in range(NCORES):
        p, b = c // 2, c % 2
        rc = res.results[c]
        out[p, b] = rc["out_u"] + fc_b[None, :] + q[b]
        attn[b, 2 * p] = rc["attnT"][0].T
        attn[b, 2 * p + 1] = rc["attnT"][1].T
    return out, attn


# revision 22
# speedup vs baseline: 1.1556x; 1.0595x over previous
"""Trainium2 Bass kernel for MPMultiHeadAttention (P=4 masks, B=2, L=1024, D=512, H=8).

Sharding: 8 cores = (p, b) pairs; core c handles mask p=c//2, batch b=c%2.
Each core computes all H=8 heads for its (p,b):
  LN(q[b]) -> Q/K/V projections -> scores^T -> exp*mask -> PV (+denominator
  via an appended ones-column on V) -> normalize -> FC.
Per-core outputs: out_u = FC output [1024,512] (host adds fc_b + residual),
attnT = pre-softmax scores, transposed, for 2 heads [2,1024,1024] (host
transposes back). Head ownership is made SPMD-uniform by permuting heads
host-side so each core's owned heads are head-slots 0 and 1.

All activations live in transposed layout (feature dim on partitions) so every
matmul contraction runs without on-device transposes; host pre-transposes
q/k/v/mask and the weights (pure layout prep on the sharded inputs).
"""

import numpy as np
import ml_dtypes

import concourse.bacc as bacc
import concourse.bass as bass
import concourse.tile as tile
from concourse import mybir
from concourse.bass_utils import run_bass_kernel_spmd

B, L, D = 2, 1024, 512
H, DK, DV, P = 8, 64, 64, 4
TEMP = float(DK) ** 0.5
EPS = 1e-6
NCORES = 8

FP32 = mybir.dt.float32
F32R = mybir.dt.float32r
BF16 = mybir.dt.bfloat16

NLT = L // 128          # 8 lq/lk tiles of 128
NDT = D // 128          # 4 d tiles of 128
NET = D // 128          # 4 e tiles (H*DK = 512)

# Profiling knobs (test.py sets TRACE=True; harness leaves it False)
TRACE = False
LAST_EXEC_NS = None
LAST_RESULTS = None

_COMPILED = None


def _emit(ctx, tc):
    nc = tc.nc
    AF = mybir.ActivationFunctionType
    OP = mybir.AluOpType

    # ---- DRAM I/O ----
    qT_d = nc.dram_tensor("qT", [D, L], F32R, kind="ExternalInput").ap()
    kT_d = nc.dram_tensor("kT", [D, L], F32R, kind="ExternalInput").ap()
    vT_d = nc.dram_tensor("vT", [D, L], F32R, kind="ExternalInput").ap()
    maskT_d = nc.dram_tensor("maskT", [L, L], BF16, kind="ExternalInput").ap()
    wqT_d = nc.dram_tensor("wqT", [D, D], F32R, kind="ExternalInput").ap()
    wkT_d = nc.dram_tensor("wkT", [D, D], F32R, kind="ExternalInput").ap()
    wvT_d = nc.dram_tensor("wvT", [D, D], F32R, kind="ExternalInput").ap()
    fcwT_d = nc.dram_tensor("fcwT", [D, D], F32R, kind="ExternalInput").ap()
    lng_d = nc.dram_tensor("ln_g", [D, 1], FP32, kind="ExternalInput").ap()
    lnb_d = nc.dram_tensor("ln_b", [D, 1], FP32, kind="ExternalInput").ap()
    onesc_d = nc.dram_tensor("ones_c", [128, 1], F32R, kind="ExternalInput").ap()
    onesr_d = nc.dram_tensor("ones_r", [1, 128], F32R, kind="ExternalInput").ap()
    outu_d = nc.dram_tensor("out_u", [L, D], FP32, kind="ExternalOutput").ap()
    attnT_d = nc.dram_tensor("attnT", [2, L, L], FP32, kind="ExternalOutput").ap()

    const = ctx.enter_context(tc.tile_pool(name="const", bufs=1))
    sb = ctx.enter_context(tc.tile_pool(name="sb", bufs=1))
    sq_pool = ctx.enter_context(tc.tile_pool(name="sqp", bufs=4))
    et_pool = ctx.enter_context(tc.tile_pool(name="etp", bufs=16))
    stage_pool = ctx.enter_context(tc.tile_pool(name="stp", bufs=3))
    fco_pool = ctx.enter_context(tc.tile_pool(name="fco", bufs=3))
    ps_big = ctx.enter_context(
        tc.tile_pool(name="psb", bufs=2, space=bass.MemorySpace.PSUM))
    ps_small = ctx.enter_context(
        tc.tile_pool(name="pss", bufs=4, space=bass.MemorySpace.PSUM))

    # ---- constants / weights ----
    wq = [const.tile([128, D], F32R, tag=f"wq{i}", name="wq") for i in range(NDT)]
    wk = [const.tile([128, D], F32R, tag=f"wk{i}", name="wk") for i in range(NDT)]
    wv = [const.tile([128, D], F32R, tag=f"wv{i}", name="wv") for i in range(NDT)]
    fcw = [const.tile([128, D], F32R, tag=f"fcw{i}", name="fcw") for i in range(NET)]
    g_col = [const.tile([128, 1], FP32, tag=f"gc{i}", name="gc") for i in range(NDT)]
    b_col = [const.tile([128, 1], FP32, tag=f"bc{i}", name="bc") for i in range(NDT)]
    ones_col = const.tile([128, 1], F32R, tag="oc")
    nc.sync.dma_start(out=ones_col, in_=onesc_d)
    ones_row = const.tile([1, 128], F32R, tag="orow")
    nc.sync.dma_start(out=ones_row, in_=onesr_d)
    eps_t = const.tile([1, 1], FP32, tag="eps")
    nc.vector.memset(eps_t, EPS)

    # ---- activations ----
    qTt = [sb.tile([128, L], F32R, tag=f"qT{i}", name="qTt") for i in range(NDT)]
    kTt = [sb.tile([128, L], F32R, tag=f"kT{i}", name="kTt") for i in range(NDT)]
    vTt = [sb.tile([128, L], F32R, tag=f"vT{i}", name="vTt") for i in range(NDT)]
    for i in range(NDT):
        for sg in range(2):
            csl = slice(sg * 512, (sg + 1) * 512)
            nc.sync.dma_start(out=qTt[i][:, csl],
                              in_=qT_d[i * 128:(i + 1) * 128, csl])
            nc.sync.dma_start(out=kTt[i][:, csl],
                              in_=kT_d[i * 128:(i + 1) * 128, csl])
            nc.sync.dma_start(out=vTt[i][:, csl],
                              in_=vT_d[i * 128:(i + 1) * 128, csl])
    maskT = [sb.tile([128, L], BF16, tag=f"mask{j}", name="maskT") for j in range(NLT)]
    for i in range(NDT):
        nc.sync.dma_start(out=g_col[i], in_=lng_d[i * 128:(i + 1) * 128, :])
        nc.sync.dma_start(out=b_col[i], in_=lnb_d[i * 128:(i + 1) * 128, :])
    for i in range(NDT):
        nc.sync.dma_start(out=wk[i], in_=wkT_d[i * 128:(i + 1) * 128, :])
        nc.sync.dma_start(out=wq[i], in_=wqT_d[i * 128:(i + 1) * 128, :])
        nc.sync.dma_start(out=wv[i], in_=wvT_d[i * 128:(i + 1) * 128, :])
    for j in range(NLT):
        nc.sync.dma_start(out=maskT[j], in_=maskT_d[j * 128:(j + 1) * 128, :])
    for i in range(NDT):
        nc.sync.dma_start(out=fcw[i], in_=fcwT_d[i * 128:(i + 1) * 128, :])

    # ---- Phase 1: LayerNorm of q, in transposed layout ----
    # col sums via ones-matmuls; mean/var rows; broadcast rows back via matmul
    mu_row = sb.tile([1, L], F32R, tag="mu")
    var_row = sb.tile([1, L], FP32, tag="var")
    rstd_row = sb.tile([1, L], F32R, tag="rstd")
    sqs = [sq_pool.tile([128, L], F32R, tag="sq", name="sq") for _ in range(NDT)]
    for i in range(NDT):
        for sg in range(2):
            csl = slice(sg * 512, (sg + 1) * 512)
            nc.vector.tensor_mul(sqs[i][:, csl], qTt[i][:, csl],
                                 qTt[i][:, csl])
    for seg in range(2):
        sl = slice(seg * 512, (seg + 1) * 512)
        ps1 = ps_small.tile([1, 512], FP32, tag="small")
        for i in range(NDT):
            nc.tensor.matmul(ps1, ones_col, qTt[i][:, sl],
                             start=(i == 0), stop=(i == NDT - 1))
        nc.scalar.mul(mu_row[:, sl], ps1, 1.0 / D)
        ps2 = ps_small.tile([1, 512], FP32, tag="small")
        for i in range(NDT):
            nc.tensor.matmul(ps2, ones_col, sqs[i][:, sl],
                             start=(i == 0), stop=(i == NDT - 1))
        nc.scalar.mul(var_row[:, sl], ps2, 1.0 / D)
    # var = E[x^2] - mu^2 ; rstd = 1/sqrt(var+eps)
    nc.vector.tensor_mul(rstd_row, mu_row, mu_row)       # rstd_row = mu^2 (tmp)
    nc.vector.tensor_sub(var_row, var_row, rstd_row)
    nc.scalar.activation(rstd_row, var_row, AF.Sqrt, bias=eps_t, scale=1.0)
    with nc.allow_low_precision(reason="f32r rstd feeds PE broadcast matmul"):
        nc.vector.reciprocal(rstd_row, rstd_row)

    mu_b = ps_big.tile([128, L], FP32, tag="big")
    rstd_b = ps_big.tile([128, L], FP32, tag="big")
    for seg in range(2):
        sl = slice(seg * 512, (seg + 1) * 512)
        nc.tensor.matmul(mu_b[:, sl], ones_row, mu_row[:, sl])
        nc.tensor.matmul(rstd_b[:, sl], ones_row, rstd_row[:, sl])
    qnT = [sb.tile([128, L], F32R, tag=f"qnT{i}", name="qnT") for i in range(NDT)]
    for i in range(NDT):
        nc.vector.tensor_sub(qnT[i], qTt[i], mu_b)
        nc.vector.tensor_mul(qnT[i], qnT[i], rstd_b)
        nc.vector.tensor_scalar(qnT[i], qnT[i], g_col[i], b_col[i],
                                op0=OP.mult, op1=OP.add)

    # ---- Phase 2: projections ----
    # kh first (khT reuses qTt slots, dead after LN), then qh (reuses kTt
    # slots), then vh (outT later reuses vTt slots).
    khT = [sb.tile([128, L], F32R, tag=f"qT{i}", name="khT") for i in range(NET)]
    for et in range(NET):
        esl = slice(et * 128, (et + 1) * 128)
        for seg in range(2):
            sl = slice(seg * 512, (seg + 1) * 512)
            psk = ps_small.tile([128, 512], FP32, tag="small")
            for kt in range(NDT):
                nc.tensor.matmul(psk, wk[kt][:, esl], kTt[kt][:, sl],
                                 start=(kt == 0), stop=(kt == NDT - 1))
            nc.scalar.copy(khT[et][:, sl], psk)
    qhT = [sb.tile([128, L], F32R, tag=f"kT{i}", name="qhT") for i in range(NET)]
    for et in range(NET):
        esl = slice(et * 128, (et + 1) * 128)
        for seg in range(2):
            sl = slice(seg * 512, (seg + 1) * 512)
            psq = ps_small.tile([128, 512], FP32, tag="small")
            for kt in range(NDT):
                nc.tensor.matmul(psq, wq[kt][:, esl], qnT[kt][:, sl],
                                 start=(kt == 0), stop=(kt == NDT - 1))
            nc.scalar.copy(qhT[et][:, sl], psq)
    # V in natural layout [lk, e], bf16, with an appended ones column per head
    vh = [sb.tile([128, H, DV + 1], BF16, tag=f"vh{i}", name="vh") for i in range(NLT)]
    for lt in range(NLT):
        lsl = slice(lt * 128, (lt + 1) * 128)
        psv = ps_small.tile([128, 512], FP32, tag="small")
        for kt in range(NDT):
            nc.tensor.matmul(psv, vTt[kt][:, lsl], wv[kt],
                             start=(kt == 0), stop=(kt == NDT - 1))
        nc.scalar.copy(
            vh[lt][:, :, 0:DV], psv.rearrange("p (h d) -> p h d", h=H))
        nc.vector.memset(vh[lt][:, :, DV:DV + 1], 1.0)

    # ---- Phase 3: attention, one head at a time for deep pipelining ----
    # Consecutive heads alternate PE rows 0-63 / 64-127, so score matmuls of
    # head h+1 are row-disjoint from head h's and overlap on the PE array.
    # PV accumulation is interleaved into the lt loop so PE work tracks the
    # ACT exp chain instead of bunching at head end.
    outT = [sb.tile([128, L], F32R, tag=f"vT{i}", name="outT") for i in range(NET)]
    for h in range(H):
        et, sub = h // 2, h % 2
        esl = slice(64 * sub, 64 * sub + 64)
        psv = [ps_small.tile([128, 512], FP32, tag="small", name="psv")
               for _ in range(2)]
        for lt in range(NLT):
            lsl = slice(lt * 128, (lt + 1) * 128)
            pst = ps_big.tile([128, L], FP32, tag="big", name="pst")
            for seg in range(2):
                sl = slice(seg * 512, (seg + 1) * 512)
                nc.tensor.matmul(pst[:, sl], khT[et][esl, lsl], qhT[et][esl, sl])
            if et == 0:  # owned heads are always slots 0/1 (host permutes)
                stg = stage_pool.tile([128, L], FP32, tag="stg", name="stg")
                nc.vector.tensor_copy(stg, pst)
                nc.sync.dma_start(out=attnT_d[sub, lt * 128:(lt + 1) * 128, :],
                                  in_=stg)
            e_t = et_pool.tile([128, L], BF16, tag="et", name="e_t")
            nc.scalar.activation(e_t, pst, AF.Exp)
            nc.vector.tensor_mul(e_t, e_t, maskT[lt])
            for seg in range(2):
                sl = slice(seg * 512, (seg + 1) * 512)
                nc.tensor.matmul(psv[seg][0:DV + 1, :], vh[lt][:, h, :],
                                 e_t[:, sl],
                                 start=(lt == 0), stop=(lt == NLT - 1),
                                 skip_group_check=True)
        den_t = sb.tile([1, L], F32R, tag="den", bufs=2, name="den")
        for seg in range(2):
            sl = slice(seg * 512, (seg + 1) * 512)
            nc.vector.tensor_copy(outT[et][esl, sl], psv[seg][0:DV, :])
            nc.vector.tensor_copy(den_t[:, sl], psv[seg][DV:DV + 1, :])
        # normalize this head: scale outT columns by 1/denom
        with nc.allow_low_precision(reason="f32r denom feeds PE broadcast matmul"):
            nc.vector.reciprocal(den_t, den_t)
        ps_s = [ps_small.tile([64, 512], FP32, tag="small", name="ps_s")
                for _ in range(2)]
        for seg in range(2):
            sl = slice(seg * 512, (seg + 1) * 512)
            nc.tensor.matmul(ps_s[seg], ones_row[0:1, 0:64], den_t[:, sl])
            nc.vector.tensor_mul(outT[et][esl, sl], outT[et][esl, sl], ps_s[seg])

    # ---- Phase 5: FC ----
    for lt in range(NLT):
        lsl = slice(lt * 128, (lt + 1) * 128)
        psf = ps_small.tile([128, 512], FP32, tag="small")
        for et in range(NET):
            nc.tensor.matmul(psf, outT[et][:, lsl], fcw[et],
                             start=(et == 0), stop=(et == NET - 1))
        fo = fco_pool.tile([128, D], FP32, tag="fo")
        nc.vector.tensor_copy(fo, psf)
        nc.scalar.dma_start(out=outu_d[lt * 128:(lt + 1) * 128, :], in_=fo)


def _build():
    global _COMPILED
    if _COMPILED is None:
        nc = bacc.Bacc("TRN2", target_bir_lowering=False, debug=False,
                       num_devices=NCORES)
        from contextlib import ExitStack
        with tile.TileContext(nc) as tc, ExitStack() as ctx:
            _emit(ctx, tc)
        nc.compile()
        _COMPILED = nc
    return _COMPILED


def kernel(q, k, v, mask, w_q, w_k, w_v, fc_w, fc_b, ln_g, ln_b):
    global LAST_EXEC_NS, LAST_RESULTS
    q = np.asarray(q, np.float32)
    k = np.asarray(k, np.float32)
    v = np.asarray(v, np.float32)
    mask = np.asarray(mask, np.float32)
    fc_b = np.asarray(fc_b, np.float32)
    ln_g = np.asarray(ln_g, np.float32)
    ln_b = np.asarray(ln_b, np.float32)
    wqT = np.ascontiguousarray(np.asarray(w_q, np.float32).T) / TEMP  # [D, H*DK]
    wkT = np.ascontiguousarray(np.asarray(w_k, np.float32).T)
    wvT = np.ascontiguousarray(np.asarray(w_v, np.float32).T)
    fcwT = np.ascontiguousarray(np.asarray(fc_w, np.float32).T)      # [H*DV, D]

    nc = _build()
    in_maps = []
    perms = []
    for c in range(NCORES):
        p, b = c // 2, c % 2
        perm = [2 * p, 2 * p + 1] + [h for h in range(H) if h not in (2 * p, 2 * p + 1)]
        perms.append(perm)
        wqT_p = np.ascontiguousarray(
            wqT.reshape(D, H, DK)[:, perm].reshape(D, D))
        wkT_p = np.ascontiguousarray(
            wkT.reshape(D, H, DK)[:, perm].reshape(D, D))
        wvT_p = np.ascontiguousarray(
            wvT.reshape(D, H, DV)[:, perm].reshape(D, D))
        fcwT_p = np.ascontiguousarray(
            fcwT.reshape(H, DV, D)[perm].reshape(D, D))
        in_maps.append({
            "qT": np.ascontiguousarray(q[b].T),
            "kT": np.ascontiguousarray(k[b].T),
            "vT": np.ascontiguousarray(v[b].T),
            "maskT": np.ascontiguousarray(mask[p, 0, 0].T).astype(ml_dtypes.bfloat16),
            "wqT": wqT_p, "wkT": wkT_p, "wvT": wvT_p, "fcwT": fcwT_p,
            "ln_g": np.ascontiguousarray(ln_g[:, None]),
            "ln_b": np.ascontiguousarray(ln_b[:, None]),
            "ones_c": np.ones((128, 1), np.float32),
            "ones_r": np.ones((1, 128), np.float32),
        })

    res = run_bass_kernel_spmd(nc, in_maps, core_ids=list(range(NCORES)),
                               trace=TRACE)
    LAST_EXEC_NS = res.exec_time_ns
    LAST_RESULTS = res

    out = np.empty((P, B, L, D), np.float32)
    attn = np.empty((B, H, L, L), np.float32)
    for c in range(NCORES):
        p, b = c // 2, c % 2
        rc = res.results[c]
        out[p, b] = rc["out_u"] + fc_b[None, :] + q[b]
        attn[b, 2 * p] = rc["attnT"][0].T
        attn[b, 2 * p + 1] = rc["attnT"][1].T
    return out, attn


# revision 24
# speedup vs baseline: 1.1557x; 1.0001x over previous
"""Trainium2 Bass kernel for MPMultiHeadAttention (P=4 masks, B=2, L=1024, D=512, H=8).

Sharding: 8 cores = (p, b) pairs; core c handles mask p=c//2, batch b=c%2.
Each core computes all H=8 heads for its (p,b):
  LN(q[b]) -> Q/K/V projections -> scores^T -> exp*mask -> PV (+denominator
  via an appended ones-column on V) -> normalize -> FC.
Per-core outputs: out_u = FC output [1024,512] (host adds fc_b + residual),
attnT = pre-softmax scores, transposed, for 2 heads [2,1024,1024] (host
transposes back). Head ownership is made SPMD-uniform by permuting heads
host-side so each core's owned heads are head-slots 0 and 1.

All activations live in transposed layout (feature dim on partitions) so every
matmul contraction runs without on-device transposes; host pre-transposes
q/k/v/mask and the weights (pure layout prep on the sharded inputs).
"""

import numpy as np
import ml_dtypes

import concourse.bacc as bacc
import concourse.bass as bass
import concourse.tile as tile
from concourse import mybir
from concourse.bass_utils import run_bass_kernel_spmd

B, L, D = 2, 1024, 512
H, DK, DV, P = 8, 64, 64, 4
TEMP = float(DK) ** 0.5
EPS = 1e-6
NCORES = 8

FP32 = mybir.dt.float32
F32R = mybir.dt.float32r
BF16 = mybir.dt.bfloat16

NLT = L // 128          # 8 lq/lk tiles of 128
NDT = D // 128          # 4 d tiles of 128
NET = D // 128          # 4 e tiles (H*DK = 512)

# Profiling knobs (test.py sets TRACE=True; harness leaves it False)
TRACE = False
LAST_EXEC_NS = None
LAST_RESULTS = None

_COMPILED = None


def _emit(ctx, tc):
    nc = tc.nc
    AF = mybir.ActivationFunctionType
    OP = mybir.AluOpType

    # ---- DRAM I/O ----
    qT_d = nc.dram_tensor("qT", [D, L], F32R, kind="ExternalInput").ap()
    kT_d = nc.dram_tensor("kT", [D, L], F32R, kind="ExternalInput").ap()
    vT_d = nc.dram_tensor("vT", [D, L], F32R, kind="ExternalInput").ap()
    maskT_d = nc.dram_tensor("maskT", [L, L], BF16, kind="ExternalInput").ap()
    wqT_d = nc.dram_tensor("wqT", [D, D], F32R, kind="ExternalInput").ap()
    wkT_d = nc.dram_tensor("wkT", [D, D], F32R, kind="ExternalInput").ap()
    wvT_d = nc.dram_tensor("wvT", [D, D], F32R, kind="ExternalInput").ap()
    fcwT_d = nc.dram_tensor("fcwT", [D, D], F32R, kind="ExternalInput").ap()
    lng_d = nc.dram_tensor("ln_g", [D, 1], FP32, kind="ExternalInput").ap()
    lnb_d = nc.dram_tensor("ln_b", [D, 1], FP32, kind="ExternalInput").ap()
    onesc_d = nc.dram_tensor("ones_c", [128, 1], F32R, kind="ExternalInput").ap()
    onesr_d = nc.dram_tensor("ones_r", [1, 128], F32R, kind="ExternalInput").ap()
    outu_d = nc.dram_tensor("out_u", [L, D], FP32, kind="ExternalOutput").ap()
    attnT_d = nc.dram_tensor("attnT", [2, L, L], FP32, kind="ExternalOutput").ap()

    const = ctx.enter_context(tc.tile_pool(name="const", bufs=1))
    sb = ctx.enter_context(tc.tile_pool(name="sb", bufs=1))
    sq_pool = ctx.enter_context(tc.tile_pool(name="sqp", bufs=4))
    et_pool = ctx.enter_context(tc.tile_pool(name="etp", bufs=14))
    stage_pool = ctx.enter_context(tc.tile_pool(name="stp", bufs=4))
    fco_pool = ctx.enter_context(tc.tile_pool(name="fco", bufs=3))
    ps_big = ctx.enter_context(
        tc.tile_pool(name="psb", bufs=2, space=bass.MemorySpace.PSUM))
    ps_small = ctx.enter_context(
        tc.tile_pool(name="pss", bufs=4, space=bass.MemorySpace.PSUM))

    # ---- constants / weights ----
    wq = [const.tile([128, D], F32R, tag=f"wq{i}", name="wq") for i in range(NDT)]
    wk = [const.tile([128, D], F32R, tag=f"wk{i}", name="wk") for i in range(NDT)]
    wv = [const.tile([128, D], F32R, tag=f"wv{i}", name="wv") for i in range(NDT)]
    fcw = [const.tile([128, D], F32R, tag=f"fcw{i}", name="fcw") for i in range(NET)]
    g_col = [const.tile([128, 1], FP32, tag=f"gc{i}", name="gc") for i in range(NDT)]
    b_col = [const.tile([128, 1], FP32, tag=f"bc{i}", name="bc") for i in range(NDT)]
    ones_col = const.tile([128, 1], F32R, tag="oc")
    nc.sync.dma_start(out=ones_col, in_=onesc_d)
    ones_row = const.tile([1, 128], F32R, tag="orow")
    nc.sync.dma_start(out=ones_row, in_=onesr_d)
    eps_t = const.tile([1, 1], FP32, tag="eps")
    nc.vector.memset(eps_t, EPS)

    # ---- activations ----
    qTt = [sb.tile([128, L], F32R, tag=f"qT{i}", name="qTt") for i in range(NDT)]
    kTt = [sb.tile([128, L], F32R, tag=f"kT{i}", name="kTt") for i in range(NDT)]
    vTt = [sb.tile([128, L], F32R, tag=f"vT{i}", name="vTt") for i in range(NDT)]
    for i in range(NDT):
        for sg in range(2):
            csl = slice(sg * 512, (sg + 1) * 512)
            nc.sync.dma_start(out=qTt[i][:, csl],
                              in_=qT_d[i * 128:(i + 1) * 128, csl])
            nc.sync.dma_start(out=kTt[i][:, csl],
                              in_=kT_d[i * 128:(i + 1) * 128, csl])
            nc.sync.dma_start(out=vTt[i][:, csl],
                              in_=vT_d[i * 128:(i + 1) * 128, csl])
    maskT = [sb.tile([128, L], BF16, tag=f"mask{j}", name="maskT") for j in range(NLT)]
    for i in range(NDT):
        nc.sync.dma_start(out=g_col[i], in_=lng_d[i * 128:(i + 1) * 128, :])
        nc.sync.dma_start(out=b_col[i], in_=lnb_d[i * 128:(i + 1) * 128, :])
    for i in range(NDT):
        nc.sync.dma_start(out=wk[i], in_=wkT_d[i * 128:(i + 1) * 128, :])
        nc.sync.dma_start(out=wq[i], in_=wqT_d[i * 128:(i + 1) * 128, :])
        nc.sync.dma_start(out=wv[i], in_=wvT_d[i * 128:(i + 1) * 128, :])
    for j in range(NLT):
        nc.sync.dma_start(out=maskT[j], in_=maskT_d[j * 128:(j + 1) * 128, :])
    for i in range(NDT):
        nc.sync.dma_start(out=fcw[i], in_=fcwT_d[i * 128:(i + 1) * 128, :])

    # ---- Phase 1: LayerNorm of q, in transposed layout ----
    # col sums via ones-matmuls; mean/var rows; broadcast rows back via matmul
    mu_row = sb.tile([1, L], F32R, tag="mu")
    var_row = sb.tile([1, L], FP32, tag="var")
    rstd_row = sb.tile([1, L], F32R, tag="rstd")
    sqs = [sq_pool.tile([128, L], F32R, tag="sq", name="sq") for _ in range(NDT)]
    for i in range(NDT):
        for sg in range(2):
            csl = slice(sg * 512, (sg + 1) * 512)
            nc.vector.tensor_mul(sqs[i][:, csl], qTt[i][:, csl],
                                 qTt[i][:, csl])
    for seg in range(2):
        sl = slice(seg * 512, (seg + 1) * 512)
        ps1 = ps_small.tile([1, 512], FP32, tag="small")
        for i in range(NDT):
            nc.tensor.matmul(ps1, ones_col, qTt[i][:, sl],
                             start=(i == 0), stop=(i == NDT - 1))
        nc.scalar.mul(mu_row[:, sl], ps1, 1.0 / D)
        ps2 = ps_small.tile([1, 512], FP32, tag="small")
        for i in range(NDT):
            nc.tensor.matmul(ps2, ones_col, sqs[i][:, sl],
                             start=(i == 0), stop=(i == NDT - 1))
        nc.scalar.mul(var_row[:, sl], ps2, 1.0 / D)
    # var = E[x^2] - mu^2 ; rstd = 1/sqrt(var+eps)
    nc.vector.tensor_mul(rstd_row, mu_row, mu_row)       # rstd_row = mu^2 (tmp)
    nc.vector.tensor_sub(var_row, var_row, rstd_row)
    nc.scalar.activation(rstd_row, var_row, AF.Sqrt, bias=eps_t, scale=1.0)
    with nc.allow_low_precision(reason="f32r rstd feeds PE broadcast matmul"):
        nc.vector.reciprocal(rstd_row, rstd_row)

    mu_b = ps_big.tile([128, L], FP32, tag="big")
    rstd_b = ps_big.tile([128, L], FP32, tag="big")
    for seg in range(2):
        sl = slice(seg * 512, (seg + 1) * 512)
        nc.tensor.matmul(mu_b[:, sl], ones_row, mu_row[:, sl])
        nc.tensor.matmul(rstd_b[:, sl], ones_row, rstd_row[:, sl])
    qnT = [sb.tile([128, L], F32R, tag=f"qnT{i}", name="qnT") for i in range(NDT)]
    for i in range(NDT):
        nc.vector.tensor_sub(qnT[i], qTt[i], mu_b)
        nc.vector.tensor_mul(qnT[i], qnT[i], rstd_b)
        nc.vector.tensor_scalar(qnT[i], qnT[i], g_col[i], b_col[i],
                                op0=OP.mult, op1=OP.add)

    # ---- Phase 2: projections ----
    # kh first (khT reuses qTt slots, dead after LN), then qh (reuses kTt
    # slots), then vh (outT later reuses vTt slots).
    khT = [sb.tile([128, L], F32R, tag=f"qT{i}", name="khT") for i in range(NET)]
    for et in range(NET):
        esl = slice(et * 128, (et + 1) * 128)
        for seg in range(2):
            sl = slice(seg * 512, (seg + 1) * 512)
            psk = ps_small.tile([128, 512], FP32, tag="small")
            for kt in range(NDT):
                nc.tensor.matmul(psk, wk[kt][:, esl], kTt[kt][:, sl],
                                 start=(kt == 0), stop=(kt == NDT - 1))
            nc.scalar.copy(khT[et][:, sl], psk)
    qhT = [sb.tile([128, L], F32R, tag=f"kT{i}", name="qhT") for i in range(NET)]
    for et in range(NET):
        esl = slice(et * 128, (et + 1) * 128)
        for seg in range(2):
            sl = slice(seg * 512, (seg + 1) * 512)
            psq = ps_small.tile([128, 512], FP32, tag="small")
            for kt in range(NDT):
                nc.tensor.matmul(psq, wq[kt][:, esl], qnT[kt][:, sl],
                                 start=(kt == 0), stop=(kt == NDT - 1))
            nc.scalar.copy(qhT[et][:, sl], psq)
    # V in natural layout [lk, e], bf16, with an appended ones column per head
    vh = [sb.tile([128, H, DV + 1], BF16, tag=f"vh{i}", name="vh") for i in range(NLT)]
    for lt in range(NLT):
        lsl = slice(lt * 128, (lt + 1) * 128)
        psv = ps_small.tile([128, 512], FP32, tag="small")
        for kt in range(NDT):
            nc.tensor.matmul(psv, vTt[kt][:, lsl], wv[kt],
                             start=(kt == 0), stop=(kt == NDT - 1))
        nc.scalar.copy(
            vh[lt][:, :, 0:DV], psv.rearrange("p (h d) -> p h d", h=H))
        nc.vector.memset(vh[lt][:, :, DV:DV + 1], 1.0)

    # ---- Phase 3: attention, one head at a time for deep pipelining ----
    # Consecutive heads alternate PE rows 0-63 / 64-127, so score matmuls of
    # head h+1 are row-disjoint from head h's and overlap on the PE array.
    # PV accumulation is interleaved into the lt loop so PE work tracks the
    # ACT exp chain instead of bunching at head end.
    outT = [sb.tile([128, L], F32R, tag=f"vT{i}", name="outT") for i in range(NET)]
    for h in range(H):
        et, sub = h // 2, h % 2
        esl = slice(64 * sub, 64 * sub + 64)
        psv = [ps_small.tile([128, 512], FP32, tag="small", name="psv")
               for _ in range(2)]
        for lt in range(NLT):
            lsl = slice(lt * 128, (lt + 1) * 128)
            pst = ps_big.tile([128, L], FP32, tag="big", name="pst")
            for seg in range(2):
                sl = slice(seg * 512, (seg + 1) * 512)
                nc.tensor.matmul(pst[:, sl], khT[et][esl, lsl], qhT[et][esl, sl])
            if et == 0:  # owned heads are always slots 0/1 (host permutes)
                stg = stage_pool.tile([128, L], FP32, tag="stg", name="stg")
                nc.vector.tensor_copy(stg, pst)
                nc.sync.dma_start(out=attnT_d[sub, lt * 128:(lt + 1) * 128, :],
                                  in_=stg)
            e_t = et_pool.tile([128, L], BF16, tag="et", name="e_t")
            nc.scalar.activation(e_t, pst, AF.Exp)
            nc.vector.tensor_mul(e_t, e_t, maskT[lt])
            for seg in range(2):
                sl = slice(seg * 512, (seg + 1) * 512)
                nc.tensor.matmul(psv[seg][0:DV + 1, :], vh[lt][:, h, :],
                                 e_t[:, sl],
                                 start=(lt == 0), stop=(lt == NLT - 1),
                                 skip_group_check=True)
        den_t = sb.tile([1, L], F32R, tag="den", bufs=2, name="den")
        for seg in range(2):
            sl = slice(seg * 512, (seg + 1) * 512)
            nc.vector.tensor_copy(outT[et][esl, sl], psv[seg][0:DV, :])
            nc.vector.tensor_copy(den_t[:, sl], psv[seg][DV:DV + 1, :])
        # normalize this head: scale outT columns by 1/denom
        with nc.allow_low_precision(reason="f32r denom feeds PE broadcast matmul"):
            nc.vector.reciprocal(den_t, den_t)
        ps_s = [ps_small.tile([64, 512], FP32, tag="small", name="ps_s")
                for _ in range(2)]
        for seg in range(2):
            sl = slice(seg * 512, (seg + 1) * 512)
            nc.tensor.matmul(ps_s[seg], ones_row[0:1, 0:64], den_t[:, sl])
            nc.vector.tensor_mul(outT[et][esl, sl], outT[et][esl, sl], ps_s[seg])

    # ---- Phase 5: FC ----
    for lt in range(NLT):
        lsl = slice(lt * 128, (lt + 1) * 128)
        psf = ps_small.tile([128, 512], FP32, tag="small")
        for et in range(NET):
            nc.tensor.matmul(psf, outT[et][:, lsl], fcw[et],
                             start=(et == 0), stop=(et == NET - 1))
        fo = fco_pool.tile([128, D], FP32, tag="fo")
        nc.vector.tensor_copy(fo, psf)
        nc.scalar.dma_start(out=outu_d[lt * 128:(lt + 1) * 128, :], in_=fo)


def _build():
    global _COMPILED
    if _COMPILED is None:
        nc = bacc.Bacc("TRN2", target_bir_lowering=False, debug=False,
                       num_devices=NCORES)
        from contextlib import ExitStack
        with tile.TileContext(nc) as tc, ExitStack() as ctx:
            _emit(ctx, tc)
        nc.compile()
        _COMPILED = nc
    return _COMPILED


def kernel(q, k, v, mask, w_q, w_k, w_v, fc_w, fc_b, ln_g, ln_b):
    global LAST_EXEC_NS, LAST_RESULTS
    q = np.asarray(q, np.float32)
    k = np.asarray(k, np.float32)
    v = np.asarray(v, np.float32)
    mask = np.asarray(mask, np.float32)
    fc_b = np.asarray(fc_b, np.float32)
    ln_g = np.asarray(ln_g, np.float32)
    ln_b = np.asarray(ln_b, np.float32)
    wqT = np.ascontiguousarray(np.asarray(w_q, np.float32).T) / TEMP  # [D, H*DK]
    wkT = np.ascontiguousarray(np.asarray(w_k, np.float32).T)
    wvT = np.ascontiguousarray(np.asarray(w_v, np.float32).T)
    fcwT = np.ascontiguousarray(np.asarray(fc_w, np.float32).T)      # [H*DV, D]

    nc = _build()
    in_maps = []
    perms = []
    for c in range(NCORES):
        p, b = c // 2, c % 2
        perm = [2 * p, 2 * p + 1] + [h for h in range(H) if h not in (2 * p, 2 * p + 1)]
        perms.append(perm)
        wqT_p = np.ascontiguousarray(
            wqT.reshape(D, H, DK)[:, perm].reshape(D, D))
        wkT_p = np.ascontiguousarray(
            wkT.reshape(D, H, DK)[:, perm].reshape(D, D))
        wvT_p = np.ascontiguousarray(
            wvT.reshape(D, H, DV)[:, perm].reshape(D, D))
        fcwT_p = np.ascontiguousarray(
            fcwT.reshape(H, DV, D)[perm].reshape(D, D))
        in_maps.append({
            "qT": np.ascontiguousarray(q[b].T),
            "kT": np.ascontiguousarray(k[b].T),
            "vT": np.ascontiguousarray(v[b].T),
            "maskT": np.ascontiguousarray(mask[p, 0, 0].T).astype(ml_dtypes.bfloat16),
            "wqT": wqT_p, "wkT": wkT_p, "wvT": wvT_p, "fcwT": fcwT_p,
            "ln_g": np.ascontiguousarray(ln_g[:, None]),
            "ln_b": np.ascontiguousarray(ln_b[:, None]),
            "ones_c": np.ones((128, 1), np.float32),
            "ones_r": np.ones((1, 128), np.float32),
        })

    res = run_bass_kernel_spmd(nc, in_maps, core_ids=list(range(NCORES)),
                               trace=TRACE)
    LAST_EXEC_NS = res.exec_time_ns
    LAST_RESULTS = res

    out = np.empty((P, B, L, D), np.float32)
    attn = np.empty((B, H, L, L), np.float32)
    for c in range(NCORES):
        p, b = c // 2, c % 2
        rc = res.results[c]
        out[p, b] = rc["out_u"] + fc_b[None, :] + q[b]
        attn[b, 2 * p] = rc["attnT"][0].T
        attn[b, 2 * p + 1] = rc["attnT"][1].T
    return out, attn
